# revision 3
# baseline (speedup 1.0000x reference)
"""Trainium2 Bass kernel for nn_Attention (B=4, T=2048, C=1024, H=16, D=64).

Sharding: core c = (batch b = c//2, head-group g = c%2).
Each core handles 1 batch x 8 heads (4 head-pairs):
  1. q/k projection of its batch's 2048 tokens (bf16 matmuls) plus a
     direct-V' projection (x tile stationary) that builds V in
     [k-token, feature] orientation — no PE transposes needed
  2. causal attention per head-pair (S^T = K @ Q^T formulation,
     unnormalized softmax; denominator via a ones-column in V'; the PV
     accumulator is evacuated to SBUF immediately so the serial
     normalization chain runs off the critical path)
  3. partial output projection over its 512 o-features -> bf16 partial [T, C]
Host sums the 2 partials per batch and adds bout (and folds the v-bias
after normalization when nonzero, since sum_k P = d cancels it).

Per-core DMA inside the timed loop: x slice 4MB (bf16) in + 4MB partial out.
"""

import os
import sys

import numpy as np

for _p in ("/opt/trn_rl_repo", "/root/.axon_site/_ro/trn_rl_repo"):
    if os.path.isdir(_p) and _p not in sys.path:
        sys.path.insert(0, _p)

import concourse.tile as tile  # noqa: E402
from concourse import bacc, mybir  # noqa: E402
from concourse.bass_utils import run_bass_kernel_spmd  # noqa: E402

B, T, C = 4, 2048, 1024
H = 16
D = C // H  # 64
NCORES = 8
HPG = 8  # heads per core (group)
NP = HPG // 2  # head-pairs per core = 4
KT = 128  # k-tile (S^T partition dim)
QB = 512  # q-block (S^T free dim)
NKT = T // KT  # 16 k-tiles
NQB = T // QB  # 4 q-blocks
TB = 512  # proj token-chunk
NCH = T // TB  # 4 chunks
NKC = C // 128  # 8 contraction tiles
NDEST = 2 * NP  # 8 q/k proj dest tiles of 128 rows (v is built directly)
SCALE = 1.0 / np.sqrt(D)
MASK_BIG = 30000.0

F32 = mybir.dt.float32
BF16 = mybir.dt.bfloat16

ALLOWED, CAUSAL, GENERAL = 0, 1, 2


def _classify_mask(mask2d):
    """Per (q-block j, k-tile kt) classification, shared across (b, h).

    mask2d: [T, T] int32, mask2d[q, k] == 0 -> masked.
    Returns (plan, genbias):
      plan[j] = list of (kt, type, aux); skipped tiles omitted.
        aux = causal offset for CAUSAL, genbias index for GENERAL.
      genbias: [n_gen, 128, 512] f32 additive bias in [k, q] orientation.
    """
    plan = [[] for _ in range(NQB)]
    gen = []
    for j in range(NQB):
        q0 = j * QB
        for kt in range(NKT):
            k0 = kt * KT
            sub = mask2d[q0 : q0 + QB, k0 : k0 + KT] != 0  # [q, k]
            if not sub.any():
                continue
            if sub.all():
                plan[j].append((kt, ALLOWED, 0))
                continue
            qi = np.arange(q0, q0 + QB)[:, None]
            ki = np.arange(k0, k0 + KT)[None, :]
            off = k0 - q0
            if off in (0, 128, 256, 384) and bool((sub == (qi >= ki)).all()):
                plan[j].append((kt, CAUSAL, off))
            else:
                bias = np.where(sub, 0.0, -MASK_BIG).astype(np.float32).T  # [k, q]
                gen.append(np.ascontiguousarray(bias))
                plan[j].append((kt, GENERAL, len(gen) - 1))
    genbias = np.stack(gen) if gen else np.zeros((1, KT, QB), np.float32)
    return plan, genbias


def _build_program(plan, n_gen, loop_n=1, phases=("proj", "attn", "out"),
                   vb_zero=True):
    """Build the single-core Bass program (identical across cores)."""
    nc = bacc.Bacc("TRN2", target_bir_lowering=False, debug=False)

    xT = nc.dram_tensor("xT", [C, T], BF16, kind="ExternalInput").ap()
    # q,k weight rows (8 x 128) and v weights in [C, f] orientation
    wqkT = nc.dram_tensor("wqkT", [C, NDEST * 128], BF16, kind="ExternalInput").ap()
    wvT = nc.dram_tensor("wvT", [C, 512], BF16, kind="ExternalInput").ap()
    bqk_s = nc.dram_tensor("bqk_s", [NDEST, 128], F32, kind="ExternalInput").ap()
    bv_s = nc.dram_tensor("bv_s", [128, NP], F32, kind="ExternalInput").ap()
    woutT = nc.dram_tensor("woutT", [4 * 128, C], BF16, kind="ExternalInput").ap()
    cmask = nc.dram_tensor("cmask", [128, 128], BF16, kind="ExternalInput").ap()
    genb = nc.dram_tensor("genb", [max(n_gen, 1), KT, QB], F32, kind="ExternalInput").ap()
    partial = nc.dram_tensor("partial", [T, C], BF16, kind="ExternalOutput").ap()

    with tile.TileContext(nc) as tc:
        _emit(tc, plan, xT, wqkT, wvT, bqk_s, bv_s, woutT, cmask, genb,
              partial, loop_n=loop_n, phases=phases, vb_zero=vb_zero)
    nc.compile()
    return nc


def _emit(tc, plan, xT, wqkT, wvT, bqk_s, bv_s, woutT, cmask, genb,
          partial, loop_n=1, phases=("proj", "attn", "out"), vb_zero=True):
    from contextlib import ExitStack

    nc = tc.nc
    ctx = ExitStack()
    const = ctx.enter_context(tc.tile_pool(name="const", bufs=1))
    xin = ctx.enter_context(tc.tile_pool(name="xin", bufs=2))
    qkv = ctx.enter_context(tc.tile_pool(name="qkv", bufs=1))
    vpp = ctx.enter_context(tc.tile_pool(name="vp", bufs=1))
    ptile_pool = ctx.enter_context(tc.tile_pool(name="ptile", bufs=6))
    small = ctx.enter_context(tc.tile_pool(name="small", bufs=4))
    evac = ctx.enter_context(tc.tile_pool(name="evac", bufs=3))
    gbuf = ctx.enter_context(tc.tile_pool(name="gbuf", bufs=2))
    # PSUM budget, 8 banks of [128, 512] f32:
    #   ps_pm: 2 (proj accumulators / V-transposes / out-proj, tags pm0/pm1)
    #   ps_s:  4 (paired-head S^T [128, 1024] x 2 bufs)
    #   ps_o:  2 (PV accumulator per head of the active pair)
    ps_pm = ctx.enter_context(tc.tile_pool(name="ps_pm", bufs=1, space="PSUM"))
    ps_s = ctx.enter_context(tc.tile_pool(name="ps_s", bufs=2, space="PSUM"))
    ps_o = ctx.enter_context(tc.tile_pool(name="ps_o", bufs=1, space="PSUM"))

    # ---- constants ----
    # tiny tensors first (bias needed by the first evacuation), then q/k
    # weights per-dest so dest-0 matmuls start after one 128KB DMA, then v.
    bias_sb = const.tile([128, NDEST], F32, tag="bias_sb")
    nc.gpsimd.dma_start(bias_sb[:], bqk_s.rearrange("m p -> p m"))
    bv_sb = const.tile([128, NP], F32, tag="bv_sb")
    nc.gpsimd.dma_start(bv_sb[:], bv_s)
    w_sb = const.tile([128, NKC, NDEST * 128], BF16, tag="w_sb")
    _wq = wqkT.rearrange("(ko p) m -> p ko m", p=128)
    for _m in range(NDEST):
        nc.gpsimd.dma_start(
            w_sb[:, :, _m * 128 : (_m + 1) * 128], _wq[:, :, _m * 128 : (_m + 1) * 128]
        )
    wv_sb = const.tile([128, NKC, 512], BF16, tag="wv_sb")
    nc.gpsimd.dma_start(wv_sb[:], wvT.rearrange("(ko p) f -> p ko f", p=128))
    wout_sb = const.tile([128, 4, C], BF16, tag="wout_sb")
    cmask_sb = const.tile([128, 128], BF16, tag="cmask_sb")

    def load_big_consts():
        nc.gpsimd.dma_start(cmask_sb[:], cmask)
        nc.gpsimd.dma_start(
            wout_sb[:], woutT.rearrange("(po p) c -> p po c", p=128)
        )

    # persistent per-batch tensors: [128, pair, T]; pair p holds heads
    # (2p, 2p+1) at partitions 0-63 / 64-127.
    qT = qkv.tile([128, NP, T], BF16, tag="qT", name="qT")
    kTt = qkv.tile([128, NP, T], BF16, tag="kT", name="kTt")
    # V' in [k-token, feature] orientation, built directly by the projection
    # (x tile as the stationary operand). Per (kt, pair, hh): 64 features
    # + a ones column at index 64 (softmax denominator trick).
    vp_all = vpp.tile([128, NKT, NP, 2, D + 1], BF16, tag="vp", name="vp")
    oT = qkv.tile([128, NP, T], BF16, tag="oT", name="oT")

    def proj_chunk(c):
        """q/k projection + direct-V' build for token chunk c."""
        tok0 = c * TB
        xch = xin.tile([128, NKC, TB], BF16, tag="xch", name="xch")
        xsrc = xT[:, tok0 : tok0 + TB].rearrange("(ko p) t -> p ko t", p=128)
        if c == 0:
            # split so the first dest's matmuls start after half the load
            nc.sync.dma_start(xch[:, 0 : NKC // 2], xsrc[:, 0 : NKC // 2])
            nc.sync.dma_start(xch[:, NKC // 2 :], xsrc[:, NKC // 2 :])
        else:
            nc.sync.dma_start(xch[:], xsrc)
        if c == 0:
            # only the ones-columns need init; v-proj fills the V region
            nc.gpsimd.memset(vp_all[:, :, :, :, D : D + 1].bitcast(BF16), 1.0)
            load_big_consts()
        yield
        # q,k dest tiles: m = 2*p + {0:q, 1:k}; alternate pm0/pm1 banks so
        # evacuation of one overlaps the next dest's matmuls.
        for m in range(NDEST):
            p_, r_ = divmod(m, 2)
            dest = (qT, kTt)[r_]
            pm = ps_pm.tile([128, TB], F32, tag=f"pm{m % 2}", name="pm")
            for kc in range(NKC):
                nc.tensor.matmul(
                    pm[:],
                    w_sb[:, kc, m * 128 : (m + 1) * 128],
                    xch[:, kc],
                    start=(kc == 0),
                    stop=(kc == NKC - 1),
                )
                if kc % 4 == 3:
                    yield
            # PSUM is readable only by DVE/ACT; during chunk 0 (pure-proj
            # phase) ACT is idle, so alternate — later chunks overlap
            # attention exp, keep them on DVE.
            if c == 0 and m % 2 == 1:
                nc.scalar.activation(
                    dest[:, p_, tok0 : tok0 + TB],
                    pm[:],
                    mybir.ActivationFunctionType.Identity,
                    bias=bias_sb[:, m : m + 1],
                )
            else:
                nc.vector.tensor_scalar_add(
                    dest[:, p_, tok0 : tok0 + TB], pm[:], bias_sb[:, m : m + 1]
                )
        # direct V': per 128-token k-tile, out[t, f] = sum_c x[c,t] Wv[f,c]
        # (x tile is the stationary operand; v-bias is folded in after
        # normalization since sum_k P = d cancels it through the softmax)
        for kt in range(c * (TB // KT), (c + 1) * (TB // KT)):
            rel = kt - c * (TB // KT)
            pv = ps_pm.tile([128, 512], F32, tag=f"pm{kt % 2}", name="pv")
            for kc in range(NKC):
                nc.tensor.matmul(
                    pv[:],
                    xch[:, kc, rel * KT : (rel + 1) * KT],
                    wv_sb[:, kc],
                    start=(kc == 0),
                    stop=(kc == NKC - 1),
                )
                if kc % 4 == 3:
                    yield
            nc.vector.tensor_copy(
                vp_all[:, kt, :, :, 0:D],
                pv[:].rearrange("p (pair hh f) -> p pair hh f", pair=NP, hh=2),
            )

    def attn_block(p_, j):
        """attention for head-pair p_, q-block j; yields after each k-tile."""
        tiles = plan[j]
        if not tiles:
            return
        n_pv = 1 if "pvone" in phases else 2  # timing probe: half the PV MMs
        o_ps = ps_o.tile([128, 2, QB], F32, tag="o01", name="o_ps")

        def emit_pv(item):
            kt_, off_, pt_, first_, last_ = item
            for hh in range(n_pv):
                nc.tensor.matmul(
                    o_ps[0 : D + 1, hh, off_:QB],
                    vp_all[:, kt_, p_, hh, :],
                    pt_[:, hh, off_:QB],
                    start=first_,
                    stop=last_,
                )

        pending = []
        for idx, (kt, typ, aux) in enumerate(tiles):
            first, last = idx == 0, idx == len(tiles) - 1
            # off = width of the fully-masked q-prefix of this tile
            off = aux if typ == CAUSAL else 0
            # emit ready PV work BEFORE this tile's S matmuls: the PE queue is
            # in-order, so a stalled S (waiting on an sp bank) must not block
            # an already-runnable PV behind it.
            if len(pending) >= 2:
                emit_pv(pending.pop(0))
            sp = ps_s.tile([128, 2, QB], F32, tag="sp", name="sp")
            for hh in range(1 if "sone" in phases else 2):
                pp = hh * D
                nc.tensor.matmul(
                    sp[:, hh, off:QB],
                    kTt[pp : pp + D, p_, kt * KT : (kt + 1) * KT],
                    qT[pp : pp + D, p_, j * QB + off : (j + 1) * QB],
                    start=True,
                    stop=True,
                )
            pt = ptile_pool.tile([128, 2, QB], BF16, tag="pt", name="pt")
            if typ == GENERAL:
                gb = gbuf.tile([128, QB], F32, tag="gb", name="gb")
                nc.sync.dma_start(gb[:], genb[aux])
                ptf = gbuf.tile([128, 2, QB], F32, tag="ptf", name="ptf")
                for hh in range(2):
                    nc.vector.tensor_add(ptf[:, hh, :], sp[:, hh, :], gb[:])
                nc.scalar.activation(
                    pt[:], ptf[:], mybir.ActivationFunctionType.Exp, scale=SCALE
                )
            else:
                nc.scalar.activation(
                    pt[:, :, off:QB],
                    sp[:, :, off:QB],
                    mybir.ActivationFunctionType.Exp,
                    scale=SCALE,
                )
                if typ == CAUSAL:
                    # zero the still-masked triangle inside the window
                    nc.vector.tensor_mul(
                        pt[:, :, off : off + 128],
                        pt[:, :, off : off + 128],
                        cmask_sb[:, None, :].to_broadcast((128, 2, 128)),
                    )
            pending.append((kt, off, pt, first, last))
            yield
        for item in pending:
            emit_pv(item)
        if "nonorm" in phases or "pvone" in phases:
            # timing probe only: skip the normalization chain
            for hh in range(n_pv):
                nc.vector.tensor_copy(
                    oT[hh * D : (hh + 1) * D, p_, j * QB : (j + 1) * QB],
                    o_ps[0:D, hh, :],
                )
            return
        # Evacuate the PV accumulator to SBUF immediately: this frees the
        # o_ps PSUM banks for the next pair's accumulation, and lets the
        # (serial, cross-engine) normalization chain below run lazily off
        # the critical path — out_block(j) only needs oT one stage later.
        osb = small.tile([D + 1, 2, QB], BF16, tag="osb", name="osb")
        nc.vector.tensor_copy(osb[:], o_ps[0 : D + 1, :, :])
        r1 = small.tile([1, 2, QB], F32, tag="r1", name="r1")
        nc.vector.reciprocal(r1[:], osb[D : D + 1, :, :])
        r64 = small.tile([D, 2, QB], F32, tag="r64", name="r64")
        nc.gpsimd.partition_broadcast(r64[:], r1[:])
        for hh in range(2):
            dst = oT[hh * D : (hh + 1) * D, p_, j * QB : (j + 1) * QB]
            nc.vector.tensor_mul(dst, osb[0:D, hh, :], r64[:, hh, :])
            if not vb_zero:
                # v-bias passes through the softmax average unchanged:
                # o = P(V + 1 bv^T)/d = PV/d + bv
                nc.vector.tensor_scalar_add(
                    dst, dst, bv_sb[hh * D : (hh + 1) * D, p_ : p_ + 1]
                )

    def out_block(j):
        """out-projection for the 512 tokens of q-block j; yields per pair."""
        for tp in range(2):  # two 256-token halves -> one 512KB DMA each
            ev = evac.tile([128, 2, C], BF16, tag="ev", name="ev")
            for ti in range(2):
                tt = 4 * j + 2 * tp + ti  # 128-token tile index
                for n in range(C // QB):
                    po = ps_pm.tile([128, QB], F32, tag=f"pm{n % 2}", name="po")
                    for p_ in range(NP):
                        nc.tensor.matmul(
                            po[:],
                            oT[:, p_, tt * 128 : (tt + 1) * 128],
                            wout_sb[:, p_, n * QB : (n + 1) * QB],
                            start=(p_ == 0),
                            stop=(p_ == NP - 1),
                        )
                    # PSUM evacuation: DVE (ACT stays exp-only while the
                    # out-projection overlaps attention stages)
                    nc.vector.tensor_copy(ev[:, ti, n * QB : (n + 1) * QB], po[:])
                    yield
            r0 = j * QB + tp * 256
            nc.sync.dma_start(
                partial[r0 : r0 + 256, :].rearrange("(ti p) c -> p ti c", p=128),
                ev[:],
            )

    def interleave(gens):
        """Round-robin-drain a list of generators."""
        gens = [g for g in gens if g is not None]
        while gens:
            gens = [g for g in gens if next(g, StopIteration) is not StopIteration]

    # ---- software pipeline ----
    loop_cm = tc.For_i(0, loop_n, 1) if loop_n > 1 else None
    if loop_cm is not None:
        loop_cm.__enter__()

    for _ in proj_chunk(0):
        pass

    if "attn" not in phases:
        for c in range(1, NCH):
            for _ in proj_chunk(c):
                pass
        dbg = evac.tile([128, QB], F32, tag="ev", name="dbg")
        nc.vector.tensor_copy(dbg[:, 0:4], qT[:, 0, 0:4].bitcast(BF16))
        nc.vector.tensor_copy(dbg[:, 4:8], kTt[:, 0, 0:4].bitcast(BF16))
        nc.vector.tensor_copy(dbg[:, 8:12], vp_all[:, 0, 0, :, 0:2].bitcast(BF16))
        nc.sync.dma_start(partial[0:1, 0:512], dbg[0:1, 0:256].bitcast(BF16))
    else:
        # stage j: attention over all pairs at q-block j, interleaved with
        # proj of chunk j+1 (tokens needed from stage j+1 onward) and the
        # out-projection of block j-1 (previous stage's finalized tokens).
        prev_out = None
        for j in range(NQB):
            n_attn = len(plan[j]) * NP
            proj = proj_chunk(j + 1) if j + 1 < NCH else None
            nproj = (
                1 + (NDEST + TB // KT) * (NKC // 4) if proj else 0
            )
            stride = max(1, round(n_attn / (nproj + 1))) if proj else 10**9
            i = 0
            gens = [attn_block(p_, j) for p_ in range(NP)]
            # run pair generators sequentially (each holds ps_o for its span)
            for g in gens:
                for _ in g:
                    i += 1
                    if proj is not None and i % stride == 0:
                        next(proj, None)
                    if prev_out is not None:
                        next(prev_out, None)
            if proj is not None:
                for _ in proj:
                    pass
            if "out" in phases:
                if prev_out is not None:
                    for _ in prev_out:
                        pass
                prev_out = out_block(j)
        if "out" in phases and prev_out is not None:
            for _ in prev_out:
                pass
        if "out" not in phases:
            dbg = evac.tile([128, QB], F32, tag="ev", name="dbg2")
            nc.vector.tensor_copy(dbg[:, 0:4], oT[:, 0, 0:4].bitcast(BF16))
            nc.sync.dma_start(partial[0:1, 0:512], dbg[0:1, 0:256].bitcast(BF16))

    if loop_cm is not None:
        loop_cm.__exit__(None, None, None)
    ctx.close()


def _prep_inputs(x, mask, Wqkv, bqkv, Wout):
    x = np.asarray(x, np.float32)
    Wqkv = np.asarray(Wqkv, np.float32)
    bqkv = np.asarray(bqkv, np.float32)
    Wout = np.asarray(Wout, np.float32)
    mask2d = np.asarray(mask).reshape(T, T)

    plan, genbias = _classify_mask(mask2d)

    import ml_dtypes

    bf = ml_dtypes.bfloat16
    cmask = np.triu(np.ones((128, 128), np.float32)).astype(bf)

    in_maps = []
    for core in range(NCORES):
        b, g = divmod(core, 2)
        xT = np.ascontiguousarray(x[b].T.astype(bf))
        h0 = g * HPG * D  # first feature row of this group's heads
        # q,k dest order m = 2*p + (q,k): rows for pair p of this group
        wrows, brows = [], []
        for p_ in range(NP):
            r0 = h0 + p_ * 128
            for sec in range(2):  # q, k sections of Wqkv
                wrows.append(Wqkv[sec * C + r0 : sec * C + r0 + 128, :])
                brows.append(bqkv[sec * C + r0 : sec * C + r0 + 128])
        wqkT = np.ascontiguousarray(np.concatenate(wrows, 0).T.astype(bf))
        bqk = np.stack(brows).astype(np.float32)
        # v weights for this group's 512 features, [C, f] orientation
        wvT = np.ascontiguousarray(
            Wqkv[2 * C + h0 : 2 * C + h0 + 512, :].T.astype(bf)
        )
        bv = bqkv[2 * C + h0 : 2 * C + h0 + 512].astype(np.float32)
        bv_s = np.ascontiguousarray(bv.reshape(NP, 128).T)  # [128, NP]
        woutT = np.ascontiguousarray(Wout[:, h0 : h0 + 512].T.astype(bf))
        in_maps.append({
            "xT": xT,
            "wqkT": wqkT,
            "wvT": wvT,
            "bqk_s": np.ascontiguousarray(bqk),
            "bv_s": bv_s,
            "woutT": woutT,
            "cmask": cmask,
            "genb": genbias,
        })
    return plan, genbias, in_maps


def run(x, mask, Wqkv, bqkv, Wout, bout, trace=False, trace_kwargs=None):
    plan, genbias, in_maps = _prep_inputs(x, mask, Wqkv, bqkv, Wout)
    vb_zero = not np.any(np.asarray(bqkv, np.float32)[2 * C :])
    nc = _build_program(plan, genbias.shape[0], vb_zero=vb_zero)
    res = run_bass_kernel_spmd(
        nc,
        in_maps,
        core_ids=list(range(NCORES)),
        trace=trace,
        **(trace_kwargs or {}),
    )
    out = np.zeros((B, T, C), np.float32)
    for b in range(B):
        out[b] = (
            res.results[2 * b]["partial"].astype(np.float32)
            + res.results[2 * b + 1]["partial"].astype(np.float32)
            + np.asarray(bout, np.float32)
        )
    return out, res


def kernel(x, mask, Wqkv, bqkv, Wout, bout):
    out, _ = run(x, mask, Wqkv, bqkv, Wout, bout, trace=False)
    return out

